# revision 1
# baseline (speedup 1.0000x reference)
"""Trainium2 Bass kernel for CascadedNN (dense_mlp).

Math (per batch row x of dim 256):
  f  = relu(x @ W1 + b1)           # 512
  f  = relu(f @ W2 + b2)           # 256
  first = sigmoid(f @ Wf + bf)
  a_t = f @ Wc[t,:256] + bc[t]     (t = 0..62)
  p_0 = first;  p_{t+1} = max(sigmoid(a_t + w_t * p_t), p_t),  w_t = Wc[t,256]
  out = [p_0, ..., p_63]           # [B, 64]

Strategy: pure data parallel over 8 cores (8192 rows each), bf16 GEMMs
with fp32 PSUM accumulation. On-chip dataflow is feature-major (x is
pre-transposed on the host). The head (first + 63 cascade feature
parts, fused into one [256, 64] weight block) runs batch-major with the
f2 activation tiles as the stationary operand, so each head matmul
lands [128 batch, 64 steps] directly in the scan layout - no transpose
or redistribution DMA. The 63-step recurrence runs as interleaved
DVE(mul-add) -> ACT(sigmoid) -> DVE(max) chains.

Batch mapping per core: row b <-> (p, f) with b = f*128 + p.
S[p, t*64 + f] holds a_t(b); O[p, f*64 + t] holds p_t(b).
"""

import numpy as np
import ml_dtypes
from contextlib import ExitStack

import concourse.bacc as bacc
import concourse.bass as bass
import concourse.mybir as mybir
from concourse import tile
from concourse.bass_utils import run_bass_kernel_spmd

BF16 = mybir.dt.bfloat16
F32 = mybir.dt.float32
AF = mybir.ActivationFunctionType
OP = mybir.AluOpType

B, D, H1, H2, T = 65536, 256, 512, 256, 64
NCORES = 8
BL = B // NCORES            # 8192 rows per core
NCHUNK = 4
CB = BL // NCHUNK           # 4096 rows per chunk
NB = CB // 512              # 512-wide psum tiles per chunk
NJ = CB // 128              # 128-row batch tiles per chunk (32)
FW = BL // 128              # 64 scan columns
FC = FW // NCHUNK           # 32 per chunk
NCH = 4                     # interleaved scan chains (== NCHUNK)
EVAC_MOD, EVAC_ACT = 3, (2,)  # evac engine rotation: 1/3 on ACT

_CACHE = {}


def _build(do_gemm=True, do_scan=True, bench_nrep=0, dve_sig=False,
           rev="r1", pool_evac=False, gp_max=False):
    nc = bacc.Bacc("TRN2", target_bir_lowering=False, debug=False,
                   num_devices=NCORES)
    # unique per-variant dummy input: defeats NEFF/executable cache
    # collisions between structurally-different builds with identical I/O
    vtag = nc.dram_tensor(
        f"vtag_g{int(do_gemm)}s{int(do_scan)}r{bench_nrep}d{int(dve_sig)}"
        f"c{NCHUNK}n{NCH}p{int(pool_evac)}m{int(gp_max)}v{rev}",
        [1, 1], F32, kind="ExternalInput")

    xt = nc.dram_tensor("xt", [2, 128, BL], BF16, kind="ExternalInput")
    w1 = nc.dram_tensor("w1", [2, 128, H1], BF16, kind="ExternalInput")
    b1 = nc.dram_tensor("b1", [4, 128, 1], F32, kind="ExternalInput")
    w2 = nc.dram_tensor("w2", [4, 128, H2], BF16, kind="ExternalInput")
    b2 = nc.dram_tensor("b2", [2, 128, 1], F32, kind="ExternalInput")
    wcat = nc.dram_tensor("wcat", [2, 128, T], BF16, kind="ExternalInput")
    bct = nc.dram_tensor("bct", [128, 512], F32, kind="ExternalInput")
    wpv = nc.dram_tensor("wpv", [128, T - 1], F32, kind="ExternalInput")
    out = nc.dram_tensor("out", [BL, T], F32, kind="ExternalOutput")

    with tile.TileContext(nc) as tc, ExitStack() as ctx:
        wpool = ctx.enter_context(tc.tile_pool(name="wts", bufs=1))
        xpool = ctx.enter_context(tc.tile_pool(name="xin", bufs=2))
        f1pool = ctx.enter_context(tc.tile_pool(name="f1", bufs=1))
        f2pool = ctx.enter_context(tc.tile_pool(name="f2", bufs=1))
        spool = ctx.enter_context(tc.tile_pool(name="sc", bufs=1))
        opool = ctx.enter_context(tc.tile_pool(name="oc", bufs=1))
        tpool = ctx.enter_context(tc.tile_pool(name="tmp", bufs=4))
        stpool = ctx.enter_context(tc.tile_pool(name="stg", bufs=3))
        pspool = ctx.enter_context(
            tc.tile_pool(name="ps", bufs=3, space=bass.MemorySpace.PSUM))

        # resident weights / constants
        w1sb = [wpool.tile([128, H1], BF16, name=f"w1_{k}", tag=f"w1_{k}")
                for k in range(2)]
        w2sb = [wpool.tile([128, H2], BF16, name=f"w2_{k}", tag=f"w2_{k}")
                for k in range(4)]
        wcsb = [wpool.tile([128, T], BF16, name=f"wc_{k}", tag=f"wc_{k}")
                for k in range(2)]
        b1sb = [wpool.tile([128, 1], F32, name=f"b1_{m}", tag=f"b1_{m}")
                for m in range(4)]
        b2sb = [wpool.tile([128, 1], F32, name=f"b2_{m}", tag=f"b2_{m}")
                for m in range(2)]
        bcsb = wpool.tile([128, 512], F32, name="bc", tag="bc")
        wpsb = wpool.tile([128, T - 1], F32, name="wp", tag="wp")
        vtsb = wpool.tile([1, 1], F32, name="vt", tag="vt")
        nc.sync.dma_start(vtsb[:], vtag[:])
        for k in range(2):
            nc.sync.dma_start(w1sb[k][:], w1[k])
        for k in range(4):
            nc.sync.dma_start(w2sb[k][:], w2[k])
            nc.gpsimd.dma_start(b1sb[k][:], b1[k])
        for k in range(2):
            nc.gpsimd.dma_start(wcsb[k][:], wcat[k])
            nc.gpsimd.dma_start(b2sb[k][:], b2[k])
        nc.gpsimd.dma_start(bcsb[:], bct[:])
        nc.gpsimd.dma_start(wpsb[:], wpv[:])

        # scan-layout buffers, one S/O pair per chain for overlap
        FS = FW // NCH
        Ss = [spool.tile([128, T * FS], BF16, name=f"S{i}", tag=f"S{i}")
              for i in range(NCH)]
        Os = [opool.tile([128, FS * T], F32, name=f"O{i}", tag=f"O{i}")
              for i in range(NCH)]
        S3s = [S[:].rearrange("p (t f) -> p t f", f=FS) for S in Ss]
        O3s = [O[:].rearrange("p (f t) -> p f t", t=T) for O in Os]
        bc3 = bcsb[:].rearrange("p (f t) -> p t f", t=T)  # [128, 64, 8]

        loop = tc.For_i(0, bench_nrep, 1) if bench_nrep else None
        if loop is not None:
            loop.__enter__()

        ev = [0]

        def evac_engine():
            e = nc.scalar if (ev[0] % EVAC_MOD) in EVAC_ACT else nc.vector
            ev[0] += 1
            return e

        def evac_bias_relu(eng, out_ap, in_ap, bias_ap):
            if eng is nc.vector:
                nc.vector.tensor_scalar(out_ap, in_ap, bias_ap, 0.0,
                                        OP.add, OP.max)
            else:
                nc.scalar.activation(out_ap, in_ap, AF.Relu, bias=bias_ap,
                                     scale=1.0)

        for c in range(NCHUNK if do_gemm else 0):
            cs = bass.ts(c, CB)
            xsb = [xpool.tile([128, CB], BF16, name=f"x{k}", tag=f"x{k}")
                   for k in range(2)]
            for k in range(2):
                nc.sync.dma_start(xsb[k][:], xt[k][:, cs])

            # L1: fT1[m] = relu(W1.T @ x + b1), feature-major bf16
            f1sb = [f1pool.tile([128, CB], BF16, name=f"f1_{m}",
                                tag=f"f1_{m}") for m in range(4)]

            def layer(nk, wsb, insb, outsb, bsb, nm):
                for m in range(len(outsb)):
                    pss = [pspool.tile([128, 512], F32, name="ps",
                                       tag="ps", bufs=6) for _ in range(NB)]
                    for k in range(nk):
                        for nb in range(NB):
                            nc.tensor.matmul(
                                pss[nb][:], wsb[k][:, bass.ts(m, 128)],
                                insb[k][:, bass.ts(nb, 512)],
                                start=(k == 0), stop=(k == nk - 1))
                    for nb in range(NB):
                        evac_bias_relu(evac_engine(),
                                       outsb[m][:, bass.ts(nb, 512)],
                                       pss[nb][:], bsb[m][:])

            layer(2, w1sb, xsb, f1sb, b1sb, "a")

            # L2: fT2[m2] = relu(W2.T @ f1 + b2)
            f2sb = [f2pool.tile([128, CB], BF16, name=f"f2_{m}",
                                tag=f"f2_{m}") for m in range(2)]
            layer(4, w2sb, f1sb, f2sb, b2sb, "b")

            # head, batch-major: for each 128-row tile j, f2_tile.T @ Wcat
            # lands [128 batch, 64 steps] in psum; 8 tiles share one bank,
            # then one strided add (+bcat) drops them into scan layout S.
            for jg in range(NJ // 8):
                psw = pspool.tile([128, 512], F32, name="psw", tag="psh",
                                  bufs=2)
                for j8 in range(8):
                    j = jg * 8 + j8
                    for k in range(2):
                        nc.tensor.matmul(
                            psw[:, bass.ts(j8, T)],
                            f2sb[k][:, bass.ts(j, 128)], wcsb[k][:],
                            start=(k == 0), stop=(k == 1))
                psv = psw[:].rearrange("p (f t) -> p t f", t=T)
                fg = c * FC + jg * 8           # global f of this group
                ch, fo = divmod(fg, FS)
                nc.vector.tensor_tensor(S3s[ch][:, :, fo:fo + 8], psv,
                                        bc3, OP.add)

        if not do_gemm:
            for i in range(NCH):
                nc.gpsimd.memset(Ss[i][:], 0.25)
        # scan: NCH interleaved chains
        if not do_scan:
            for i in range(NCH):
                nc.vector.tensor_copy(O3s[i][:, :, :],
                                      S3s[i][:].rearrange("p t f -> p f t"))
        for i in range(NCH if do_scan else 0):
            nc.scalar.activation(O3s[i][:, :, 0], S3s[i][:, 0, :],
                                 AF.Sigmoid)
        zt = {}
        sg = {}
        for t in range(1 if do_scan else T, T):
            for i in range(NCH):
                zt[i] = tpool.tile([128, FS], F32, name=f"z{i}", tag=f"z{i}")
                nc.vector.scalar_tensor_tensor(
                    zt[i][:], O3s[i][:, :, t - 1], wpsb[:, t - 1:t],
                    S3s[i][:, t, :], OP.mult, OP.add)
            for i in range(NCH):
                sg[i] = tpool.tile([128, FS], F32, name=f"s{i}", tag=f"s{i}")
                if dve_sig:   # bench-only: fake sigmoid on DVE
                    nc.vector.tensor_scalar(sg[i][:], zt[i][:], 0.25, 0.5,
                                            OP.mult, OP.add)
                else:
                    nc.scalar.activation(sg[i][:], zt[i][:], AF.Sigmoid)
            for i in range(NCH):
                eng = nc.gpsimd if gp_max else nc.vector
                eng.tensor_tensor(O3s[i][:, :, t], sg[i][:],
                                  O3s[i][:, :, t - 1], OP.max)

        # output: out[f*128 + p, t] = O[p, f*64 + t]
        ov = out[:].rearrange("(f p) t -> p f t", p=128)
        for i in range(NCH):
            nc.sync.dma_start(ov[:, bass.ts(i, FS), :], O3s[i][:, :, :])

        if loop is not None:
            loop.__exit__(None, None, None)

    nc.compile()
    return nc


def _prep_shared(W1, b1, W2, b2, Wf, bf, Wc, bc):
    bf16 = ml_dtypes.bfloat16
    f32 = np.float32
    W1 = np.asarray(W1, f32)
    W2 = np.asarray(W2, f32)
    Wf = np.asarray(Wf, f32)
    Wc = np.asarray(Wc, f32)
    d = {}
    d["w1"] = np.ascontiguousarray(W1.astype(bf16).reshape(2, 128, H1))
    d["w2"] = np.ascontiguousarray(W2.astype(bf16).reshape(4, 128, H2))
    wcat = np.concatenate([Wf, Wc[:, :H2].T], axis=1)   # [256, 64]
    d["wcat"] = np.ascontiguousarray(wcat.astype(bf16).reshape(2, 128, T))
    d["b1"] = np.ascontiguousarray(np.asarray(b1, f32).reshape(4, 128, 1))
    d["b2"] = np.ascontiguousarray(np.asarray(b2, f32).reshape(2, 128, 1))
    bcat = np.concatenate([np.asarray(bf, f32), np.asarray(bc, f32)])
    d["bct"] = np.ascontiguousarray(
        np.tile(bcat, (128, 8)).astype(f32))            # [128, 8*64]
    d["wpv"] = np.ascontiguousarray(
        np.broadcast_to(Wc[:, H2], (128, T - 1)).astype(f32))
    return d


def _core_inputs(x, shared, c):
    bf16 = ml_dtypes.bfloat16
    xs = x[c * BL:(c + 1) * BL, :]
    m = dict(shared)
    m["xt"] = np.ascontiguousarray(xs.T.astype(bf16)).reshape(2, 128, BL)
    return m


def kernel(x, W1, b1, W2, b2, Wf, bf, Wc, bc):
    if "nc" not in _CACHE:
        _CACHE["nc"] = _build()
    nc = _CACHE["nc"]

    x = np.asarray(x, np.float32)
    shared = _prep_shared(W1, b1, W2, b2, Wf, bf, Wc, bc)
    in_maps = [_core_inputs(x, shared, c) for c in range(NCORES)]

    # zero-fill any declared inputs we don't feed (e.g. the variant tag)
    pname = nc.partition_id_tensor.name if nc.partition_id_tensor else None
    for alloc in nc.m.functions[0].allocations:
        if (isinstance(alloc, mybir.MemoryLocationSet)
                and alloc.kind == "ExternalInput"):
            nm = alloc.memorylocations[0].name
            if nm != pname:
                for m in in_maps:
                    if nm not in m:
                        m[nm] = np.zeros(tuple(alloc.tensor_shape),
                                         mybir.dt.np(alloc.dtype))

    res = run_bass_kernel_spmd(nc, in_maps, list(range(NCORES)))
    outs = [np.asarray(res.results[c]["out"], np.float32)
            for c in range(NCORES)]
    return np.concatenate(outs, axis=0)

